# revision 8
# baseline (speedup 1.0000x reference)
"""Trainium2 Bass kernel for nn_HPN_65369402245667 (HPN pointer-network step).

Strategy: pure data-parallel over batch B=512 -> 64 rows per NeuronCore x 8.
On-device layout is transposed (D=128 on partitions, rows on the free dim) so
every linear layer is a single K=128 matmul and biases are per-partition
scalars.  Host pre-transposes inputs/weights and folds r / (1-r) / scale
constants into the weights.  Big matmuls run as float32r (1 cyc/row); the
tiny LSTM runs plain fp32.  The two Bahdanau v-dots are expressed as M=64
one-hot-column matmuls accumulating straight into two persistent PSUM tiles,
so u lands with b on partitions and softmax is a handful of wide ops.
"""
import sys, os
sys.path.insert(0, '/opt/trn_rl_repo')
import numpy as np
from contextlib import ExitStack

import concourse.bass as bass
import concourse.tile as tile
from concourse import bacc, mybir
from concourse import bass_utils

dt = mybir.dt
AF = mybir.ActivationFunctionType
ALU = mybir.AluOpType

B, S, D = 512, 1000, 128
NCORES = 8
B_LOC = B // NCORES          # 64
TILE = 500                   # rows per matmul tile (<=512 for f32 PSUM bank)
TPB = S // TILE              # tiles per batch element = 2


def build(mm_tag="f32r", b_loc=B_LOC):
    """Build + bacc-compile the per-core program. mm_tag in {f32r, f32}."""
    mmdt = dt.float32r if mm_tag == "f32r" else dt.float32
    f32 = dt.float32
    nc = bacc.Bacc("TRN2", target_bir_lowering=False, debug=False)

    def din(name, shape, dtype=f32):
        return nc.dram_tensor(name, shape, dtype, kind="ExternalInput").ap()

    def dout(name, shape, dtype=f32):
        return nc.dram_tensor(name, shape, dtype, kind="ExternalOutput").ap()

    tcT   = din("tcT", (D, b_loc * S), mmdt)
    xaT   = din("xaT", (b_loc, 3, S), mmdt)
    maskd = din("mask", (b_loc, S))
    xT    = din("xT", (2, b_loc))
    hT    = din("hT", (D, b_loc))
    cT    = din("cT", (D, b_loc))
    embxT = din("embxT", (2, D))
    lstmW = din("lstmW", (D, 13 * D))   # [Wx0-3 | Wh0-3 | Wc0-2 | Wq1 | Wq2] each .T
    biasP = din("biasP", (D, 13))
    embT  = din("embT", (3, D), mmdt)
    WmixT = din("WmixT", (D, 3 * D), mmdt)
    aggWT = din("aggWT", (D, 3 * D), mmdt)
    pWrefT = din("pWrefT", (D, D), mmdt)
    tWrefT = din("tWrefT", (D, D), mmdt)
    vst1  = din("vst1", (b_loc, D, b_loc), mmdt)
    vst2  = din("vst2", (b_loc, D, b_loc), mmdt)
    ident = din("ident", (D, D))

    probs  = dout("probs", (b_loc, S))
    latent = dout("latent", (b_loc, S))
    hnew   = dout("hnew", (b_loc, D))
    cnew   = dout("cnew", (b_loc, D))

    with tile.TileContext(nc) as tc, ExitStack() as ctx:
        cons = ctx.enter_context(tc.tile_pool(name="cons", bufs=1))

        # ---- persistent constants in SBUF ----
        embT_s = cons.tile([3, D], mmdt)
        nc.sync.dma_start(out=embT_s, in_=embT)
        WmixT_s = cons.tile([D, 3 * D], mmdt)
        nc.sync.dma_start(out=WmixT_s, in_=WmixT)
        aggWT_s = cons.tile([D, 3 * D], mmdt)
        nc.sync.dma_start(out=aggWT_s, in_=aggWT)
        pWrefT_s = cons.tile([D, D], mmdt)
        nc.sync.dma_start(out=pWrefT_s, in_=pWrefT)
        tWrefT_s = cons.tile([D, D], mmdt)
        nc.sync.dma_start(out=tWrefT_s, in_=tWrefT)
        lstmW_s = cons.tile([D, 13 * D], f32)
        nc.sync.dma_start(out=lstmW_s, in_=lstmW)
        biasP_s = cons.tile([D, 13], f32)
        nc.sync.dma_start(out=biasP_s, in_=biasP)
        embxT_s = cons.tile([2, D], f32)
        nc.sync.dma_start(out=embxT_s, in_=embxT)
        ident_s = cons.tile([D, D], f32)
        nc.sync.dma_start(out=ident_s, in_=ident)
        mask_s = cons.tile([b_loc, S], f32)
        nc.sync.dma_start(out=mask_s, in_=maskd)
        xT_s = cons.tile([2, b_loc], f32)
        nc.sync.dma_start(out=xT_s, in_=xT)
        hT_s = cons.tile([D, b_loc], f32)
        nc.sync.dma_start(out=hT_s, in_=hT)
        cT_s = cons.tile([D, b_loc], f32)
        nc.sync.dma_start(out=cT_s, in_=cT)
        qq1_s = cons.tile([D, b_loc], f32)
        qq2_s = cons.tile([D, b_loc], f32)

        def Wl(i):  # i-th 128-col block of lstmW
            return lstmW_s[:, i * D:(i + 1) * D]

        # ---- LSTM cell (all fp32; tiny: N=b_loc) ----
        with tc.tile_pool(name="lps", bufs=3, space="PSUM") as lps, \
             tc.tile_pool(name="lsb", bufs=1) as lsb:
            ps_xe = lps.tile([D, b_loc], f32, tag="ps")
            nc.tensor.matmul(ps_xe[:], embxT_s[:], xT_s[:], start=True, stop=True)
            xe_s = lsb.tile([D, b_loc], f32)
            nc.scalar.copy(xe_s[:], ps_xe[:])

            def gate(psname, blocks, rhss, bias_col, func, outtile):
                ps = lps.tile([D, b_loc], f32, tag="ps")
                n = len(blocks)
                for j, (wb, rh) in enumerate(zip(blocks, rhss)):
                    nc.tensor.matmul(ps[:], wb, rh[:], start=(j == 0), stop=(j == n - 1))
                nc.scalar.activation(outtile[:], ps[:], func,
                                     bias=biasP_s[:, bias_col:bias_col + 1])

            ig_s = lsb.tile([D, b_loc], f32)
            gate("i", [Wl(0), Wl(4), Wl(8)], [xe_s, hT_s, cT_s], 7, AF.Sigmoid, ig_s)
            fg_s = lsb.tile([D, b_loc], f32)
            gate("f", [Wl(1), Wl(5), Wl(9)], [xe_s, hT_s, cT_s], 8, AF.Sigmoid, fg_s)
            gt_s = lsb.tile([D, b_loc], f32)
            gate("g", [Wl(2), Wl(6)], [xe_s, hT_s], 9, AF.Tanh, gt_s)

            tfc = lsb.tile([D, b_loc], f32)
            nc.vector.tensor_mul(tfc[:], fg_s[:], cT_s[:])
            tig = lsb.tile([D, b_loc], f32)
            nc.vector.tensor_mul(tig[:], ig_s[:], gt_s[:])
            cnewT_s = lsb.tile([D, b_loc], f32)
            nc.vector.tensor_add(cnewT_s[:], tfc[:], tig[:])

            og_s = lsb.tile([D, b_loc], f32)
            gate("o", [Wl(3), Wl(7), Wl(10)], [xe_s, hT_s, cnewT_s], 10, AF.Sigmoid, og_s)
            tc_t = lsb.tile([D, b_loc], f32)
            nc.scalar.activation(tc_t[:], cnewT_s[:], AF.Tanh)
            hnewT_s = lsb.tile([D, b_loc], f32)
            nc.vector.tensor_mul(hnewT_s[:], og_s[:], tc_t[:])

            ps_q1 = lps.tile([D, b_loc], f32, tag="ps")
            nc.tensor.matmul(ps_q1[:], Wl(11), hnewT_s[:], start=True, stop=True)
            nc.vector.tensor_scalar_add(qq1_s[:], ps_q1[:], biasP_s[:, 11:12])
            ps_q2 = lps.tile([D, b_loc], f32, tag="ps")
            nc.tensor.matmul(ps_q2[:], Wl(12), hnewT_s[:], start=True, stop=True)
            nc.vector.tensor_scalar_add(qq2_s[:], ps_q2[:], biasP_s[:, 12:13])

            # transpose h_new / c_new back to row layout and store
            ps_hr = lps.tile([b_loc, D], f32, tag="tr")
            nc.tensor.transpose(ps_hr[:], hnewT_s[:, :b_loc], ident_s[:])
            hr_s = lsb.tile([b_loc, D], f32)
            nc.scalar.copy(hr_s[:], ps_hr[:])
            nc.sync.dma_start(out=hnew, in_=hr_s[:])
            ps_cr = lps.tile([b_loc, D], f32, tag="tr")
            nc.tensor.transpose(ps_cr[:], cnewT_s[:, :b_loc], ident_s[:])
            cr_s = lsb.tile([b_loc, D], f32)
            nc.scalar.copy(cr_s[:], ps_cr[:])
            nc.sync.dma_start(out=cnew, in_=cr_s[:])

        # ---- main per-tile pipeline ----
        p_tc = ctx.enter_context(tc.tile_pool(name="p_tc", bufs=3))
        p_xa = ctx.enter_context(tc.tile_pool(name="p_xa", bufs=3))
        p_ctx = ctx.enter_context(tc.tile_pool(name="p_ctx", bufs=6))
        p_A = ctx.enter_context(tc.tile_pool(name="p_A", bufs=3))
        p_t = ctx.enter_context(tc.tile_pool(name="p_t", bufs=3))
        p_v = ctx.enter_context(tc.tile_pool(name="p_v", bufs=2))
        p_ps = ctx.enter_context(tc.tile_pool(name="p_ps", bufs=5, space="PSUM"))
        p_U = ctx.enter_context(tc.tile_pool(name="p_U", bufs=1, space="PSUM"))
        sb = ctx.enter_context(tc.tile_pool(name="sb", bufs=1))

        psU = [p_U.tile([b_loc, TILE], f32, tag=f"U{t}", name=f"psU{t}")
               for t in range(TPB)]

        for b in range(b_loc):
            v1_s = p_v.tile([D, b_loc], mmdt, tag="v1")
            nc.sync.dma_start(out=v1_s, in_=vst1[b])
            v2_s = p_v.tile([D, b_loc], mmdt, tag="v2")
            nc.sync.dma_start(out=v2_s, in_=vst2[b])
            for t in range(TPB):
                off = t * TILE
                tc_s = p_tc.tile([D, TILE], mmdt, tag="tc")
                nc.sync.dma_start(out=tc_s, in_=tcT[:, b * S + off: b * S + off + TILE])
                xa_s = p_xa.tile([3, TILE], mmdt, tag="xa")
                nc.sync.dma_start(out=xa_s, in_=xaT[b, :, off:off + TILE])

                pe = p_ps.tile([D, TILE], f32, tag="mm")
                nc.tensor.matmul(pe[:], embT_s[:], xa_s[:], start=True, stop=True)
                prev = p_ctx.tile([D, TILE], mmdt, tag="ctx")
                nc.vector.tensor_scalar_add(prev[:], pe[:], biasP_s[:, 6:7])

                for i in range(3):
                    pmw = p_ps.tile([D, TILE], f32, tag="mm")
                    nc.tensor.matmul(pmw[:], WmixT_s[:, i * D:(i + 1) * D], prev[:],
                                     start=True, stop=True)
                    pma = p_ps.tile([D, TILE], f32, tag="mm")
                    nc.tensor.matmul(pma[:], aggWT_s[:, i * D:(i + 1) * D], prev[:],
                                     start=True, stop=True)
                    A = p_A.tile([D, TILE], f32, tag="A")
                    nc.scalar.activation(A[:], pma[:], AF.Relu,
                                         bias=biasP_s[:, 3 + i:4 + i])
                    nxt = p_ctx.tile([D, TILE], mmdt, tag="ctx")
                    nc.vector.scalar_tensor_tensor(
                        nxt[:], pmw[:], biasP_s[:, i:i + 1], A[:],
                        op0=ALU.add, op1=ALU.add)
                    prev = nxt

                prr1 = p_ps.tile([D, TILE], f32, tag="mm")
                nc.tensor.matmul(prr1[:], pWrefT_s[:], prev[:], start=True, stop=True)
                t1 = p_t.tile([D, TILE], mmdt, tag="t1")
                nc.scalar.activation(t1[:], prr1[:], AF.Tanh, bias=qq1_s[:, b:b + 1])
                nc.tensor.matmul(psU[t][:], v1_s[:], t1[:],
                                 start=(b == 0), stop=False, skip_group_check=True)

                prr2 = p_ps.tile([D, TILE], f32, tag="mm")
                nc.tensor.matmul(prr2[:], tWrefT_s[:], tc_s[:], start=True, stop=True)
                t2 = p_t.tile([D, TILE], mmdt, tag="t2")
                nc.scalar.activation(t2[:], prr2[:], AF.Tanh, bias=qq2_s[:, b:b + 1])
                nc.tensor.matmul(psU[t][:], v2_s[:], t2[:],
                                 start=False, stop=(b == b_loc - 1), skip_group_check=True)

        # ---- softmax over S per row ----
        U_s = sb.tile([b_loc, S], f32)
        for t in range(TPB):
            nc.scalar.copy(U_s[:, t * TILE:(t + 1) * TILE], psU[t][:])
        nc.sync.dma_start(out=latent, in_=U_s[:])

        th_s = sb.tile([b_loc, S], f32)
        nc.scalar.activation(th_s[:], U_s[:], AF.Tanh)
        L_s = sb.tile([b_loc, S], f32)
        nc.vector.scalar_tensor_tensor(L_s[:], th_s[:], 100.0, mask_s[:],
                                       op0=ALU.mult, op1=ALU.add)
        mx_s = sb.tile([b_loc, 1], f32)
        nc.vector.reduce_max(mx_s[:], L_s[:], axis=mybir.AxisListType.X)
        nmx_s = sb.tile([b_loc, 1], f32)
        nc.vector.tensor_scalar_mul(nmx_s[:], mx_s[:], -1.0)
        ex_s = sb.tile([b_loc, S], f32)
        sum_s = sb.tile([b_loc, 1], f32)
        nc.scalar.activation(ex_s[:], L_s[:], AF.Exp, bias=nmx_s[:, 0:1],
                             accum_out=sum_s[:])
        rec_s = sb.tile([b_loc, 1], f32)
        nc.vector.reciprocal(rec_s[:], sum_s[:])
        pr_s = sb.tile([b_loc, S], f32)
        nc.vector.tensor_scalar_mul(pr_s[:], ex_s[:], rec_s[:, 0:1])
        nc.sync.dma_start(out=probs, in_=pr_s[:])

    nc.compile()
    return nc


# ---------------- host side ----------------

def _prep(inputs):
    """Host preprocessing -> list of 8 per-core input maps."""
    f = lambda k: np.asarray(inputs[k], dtype=np.float32)
    Tc = f('Transcontext'); x = f('x'); X_all = f('X_all'); mask = f('mask')
    h = f('h'); c = f('c')
    emb_x_w = f('emb_x_w'); emb_x_b = f('emb_x_b')
    emb_all2_w = f('emb_all2_w'); emb_all2_b = f('emb_all2_b')
    W_w = f('W_w'); W_b = f('W_b'); agg_w = f('agg_w'); agg_b = f('agg_b')
    r = f('r')
    lstm_Wx = f('lstm_Wx'); lstm_bx = f('lstm_bx')
    lstm_Wh = f('lstm_Wh'); lstm_bh = f('lstm_bh')
    lstm_Wc = f('lstm_Wc'); lstm_bc = f('lstm_bc')
    ptr_v = f('ptr_v'); ptr_Wq_w = f('ptr_Wq_w'); ptr_Wq_b = f('ptr_Wq_b')
    ptr_Wref_w = f('ptr_Wref_w'); ptr_Wref_b = f('ptr_Wref_b')
    tptr_v = f('tptr_v'); tptr_Wq_w = f('tptr_Wq_w'); tptr_Wq_b = f('tptr_Wq_b')
    tptr_Wref_w = f('tptr_Wref_w'); tptr_Wref_b = f('tptr_Wref_b')

    delta = X_all - x[:, None, :]
    dist = np.sqrt(delta[..., 0] ** 2 + delta[..., 1] ** 2)[..., None]
    Xa = np.concatenate([dist, delta], axis=2)            # (B,S,3)

    scale = 1.0 / (S - 1)
    WmixT = np.concatenate([(r[i] * W_w[i]).T for i in range(3)], axis=1)
    aggWT = np.concatenate([(((1 - r[i]) * scale) * agg_w[i]).T for i in range(3)], axis=1)
    embT = np.ascontiguousarray(emb_all2_w.T)
    pWrefT = np.ascontiguousarray(ptr_Wref_w.T)
    tWrefT = np.ascontiguousarray(tptr_Wref_w.T)
    lstmW = np.concatenate(
        [lstm_Wx[g].T for g in range(4)] + [lstm_Wh[g].T for g in range(4)]
        + [lstm_Wc[g].T for g in range(3)] + [ptr_Wq_w.T, tptr_Wq_w.T], axis=1)

    biasP = np.zeros((D, 13), np.float32)
    for i in range(3):
        biasP[:, i] = r[i] * W_b[i]
        biasP[:, 3 + i] = (1 - r[i]) * agg_b[i]
    biasP[:, 6] = emb_all2_b
    wxb = lstm_Wx @ emb_x_b                               # (4,128)
    biasP[:, 7] = lstm_bx[0] + lstm_bh[0] + lstm_bc[0] + wxb[0]
    biasP[:, 8] = lstm_bx[1] + lstm_bh[1] + lstm_bc[1] + wxb[1]
    biasP[:, 9] = lstm_bx[2] + lstm_bh[2] + wxb[2]
    biasP[:, 10] = lstm_bx[3] + lstm_bh[3] + lstm_bc[2] + wxb[3]
    biasP[:, 11] = ptr_Wq_b + ptr_Wref_b
    biasP[:, 12] = tptr_Wq_b + tptr_Wref_b

    vst1 = np.zeros((B_LOC, D, B_LOC), np.float32)
    vst1[np.arange(B_LOC), :, np.arange(B_LOC)] = ptr_v[None, :]
    vst2 = np.zeros((B_LOC, D, B_LOC), np.float32)
    vst2[np.arange(B_LOC), :, np.arange(B_LOC)] = tptr_v[None, :]
    ident = np.eye(D, dtype=np.float32)
    embxT = np.ascontiguousarray(emb_x_w.T)

    shared = dict(embxT=embxT, lstmW=np.ascontiguousarray(lstmW),
                  biasP=biasP, embT=embT,
                  WmixT=np.ascontiguousarray(WmixT),
                  aggWT=np.ascontiguousarray(aggWT),
                  pWrefT=pWrefT, tWrefT=tWrefT,
                  vst1=vst1, vst2=vst2, ident=ident)

    in_maps = []
    for cix in range(NCORES):
        bs = slice(cix * B_LOC, (cix + 1) * B_LOC)
        rs = slice(cix * B_LOC * S, (cix + 1) * B_LOC * S)
        m = dict(shared)
        m['tcT'] = np.ascontiguousarray(Tc[rs].T)
        m['xaT'] = np.ascontiguousarray(Xa[bs].transpose(0, 2, 1))
        m['mask'] = np.ascontiguousarray(mask[bs])
        m['xT'] = np.ascontiguousarray(x[bs].T)
        m['hT'] = np.ascontiguousarray(h[bs].T)
        m['cT'] = np.ascontiguousarray(c[bs].T)
        in_maps.append(m)
    return in_maps, Tc


_NC_CACHE = {}

def _get_nc(mm_tag):
    if mm_tag not in _NC_CACHE:
        _NC_CACHE[mm_tag] = build(mm_tag)
    return _NC_CACHE[mm_tag]


MM_TAG = os.environ.get("HPN_MM_DTYPE", "f32r")

def run(inputs, trace=False, mm_tag=None):
    mm_tag = mm_tag or MM_TAG
    nc = _get_nc(mm_tag)
    in_maps, Tc = _prep(inputs)
    res = bass_utils.run_bass_kernel_spmd(
        nc, in_maps, core_ids=list(range(NCORES)), trace=False)
    probs = np.concatenate([res.results[cix]['probs'] for cix in range(NCORES)], axis=0)
    latent = np.concatenate([res.results[cix]['latent'] for cix in range(NCORES)], axis=0)
    hnew = np.concatenate([res.results[cix]['hnew'] for cix in range(NCORES)], axis=0)
    cnew = np.concatenate([res.results[cix]['cnew'] for cix in range(NCORES)], axis=0)
    return (Tc, probs, hnew, cnew, latent), res


def kernel(**inputs):
    out, _ = run(inputs)
    return out


def time_exec(inputs, iters=8, mm_tag=None):
    """Time device-side execution with inputs pre-placed on the 8 cores."""
    import time as _time
    import jax
    from jax.sharding import Mesh, PartitionSpec, NamedSharding
    from jax.experimental.shard_map import shard_map
    from concourse import bass2jax, mybir as _mb

    mm_tag = mm_tag or MM_TAG
    nc = _get_nc(mm_tag)
    in_maps, _ = _prep(inputs)
    bass2jax.install_neuronx_cc_hook()

    in_names, out_names, out_avals, zero_outs = [], [], [], []
    for alloc in nc.m.functions[0].allocations:
        if not isinstance(alloc, _mb.MemoryLocationSet):
            continue
        name = alloc.memorylocations[0].name
        pname = nc.partition_id_tensor.name if nc.partition_id_tensor else None
        if alloc.kind == "ExternalInput":
            if name != pname:
                in_names.append(name)
        elif alloc.kind == "ExternalOutput":
            out_names.append(name)
            out_avals.append(jax.core.ShapedArray(
                tuple(alloc.tensor_shape), _mb.dt.np(alloc.dtype)))
            zero_outs.append(np.zeros(tuple(alloc.tensor_shape),
                                      _mb.dt.np(alloc.dtype)))
    n_params = len(in_names)
    all_names = in_names + out_names
    pname = nc.partition_id_tensor.name if nc.partition_id_tensor else None
    if pname is not None:
        all_names = all_names + [pname]

    def _body(*args):
        operands = list(args)
        if pname is not None:
            operands.append(bass2jax.partition_id_tensor())
        outs = bass2jax._bass_exec_p.bind(
            *operands, out_avals=tuple(out_avals), in_names=tuple(all_names),
            out_names=tuple(out_names), lowering_input_output_aliases=(),
            sim_require_finite=True, sim_require_nnan=True, nc=nc)
        return tuple(outs)

    devices = jax.devices()[:NCORES]
    mesh = Mesh(np.asarray(devices), ("core",))
    spec = PartitionSpec("core")
    sharded = jax.jit(shard_map(
        _body, mesh=mesh, in_specs=(spec,) * (n_params + len(out_names)),
        out_specs=(spec,) * len(out_names), check_rep=False),
        keep_unused=True)
    sh = NamedSharding(mesh, spec)
    concat_in = [jax.device_put(
        np.concatenate([np.asarray(in_maps[c][nm]) for c in range(NCORES)], axis=0), sh)
        for nm in in_names]
    concat_z = [jax.device_put(
        np.zeros((NCORES * z.shape[0], *z.shape[1:]), z.dtype), sh)
        for z in zero_outs]
    out = sharded(*concat_in, *concat_z)
    jax.block_until_ready(out)
    ts = []
    for _ in range(iters):
        t0 = _time.perf_counter()
        out = sharded(*concat_in, *concat_z)
        jax.block_until_ready(out)
        ts.append(_time.perf_counter() - t0)
    return min(ts), ts
